# revision 27
# baseline (speedup 1.0000x reference)
"""Trainium2 Bass kernel for nn_BRNNIntegrateOnehot.

Reference computation (per batch b):
    h = one_hot(0, S)
    for t in 0..L-1:
        h = clip(h @ fsa[input[b, t]], -10.0, 10.0)
        out[b, t, :] = h

Data-parallel over batch B across 8 cores (8 sequences each), fsa
replicated per core in HBM as a bf16 table.  The dominant cost is the
per-(b, t) 32KB matrix gather (2.1GB of table traffic total).  Gathers
run as *static pair ops*: token ids are known on the host when kernel()
is called, so the program bakes the byte offsets in, coalescing two
stream-adjacent matrices into one 3-dim-AP HWDGE dma_start (any two
addresses form a 2-element stride); this amortizes the ~0.6us per-op
fixed cost of dynamic-HWDGE descriptor generation that limits
one-matrix-per-op designs.  Ops are split across the SP and ACT engines
(two parallel HW-DGE rings, ~1.85x).  Since each core's token stream
differs, the single SPMD program carries one gather stream per core,
branched on partition_id() (all branches have identical semaphore/slot
schedules, so the shared PE/DVE programs work for every core).

The mat-vec is one bf16 PE matmul per lane (lhsT = T so h stays a
[128,1] column; FWL halves weight-load time vs f32), clip is a fused
max/min tensor_scalar on DVE writing bf16, and the h history is
transposed band-by-band with DVE 32x32 block transposes trickled one per
step.  Output stores drain on SP after the gather stream ends (stbuf has
one slot per output group, so DVE never blocks on them).

Output returns as bf16 and is upcast on the host; the
max|err|/max|expected| metric is ~2e-4, dominated by bf16 rounding.

Raw bass (explicit engine programs + semaphores).  Self-contained.
"""

import numpy as np

V, S = 10000, 128
B, L = 64, 512
N_CORES = 8
B_LOC = B // N_CORES  # 8

TBYTES = 2             # fsa table element size: 2 = bf16, 1 = fp8 e4m3
MAT_BYTES = S * S * TBYTES
ROW_BYTES = S * TBYTES


def build_kernel(offs_cores, l=L, b_loc=B_LOC, v=V, g_slots=512, tsz=128,
                 instrument=False, tick_cyc=12000, maxtick=256):
    """offs_cores: [n_cores][l*b_loc] int byte offsets into the table,
    stream order n = t*b_loc + b."""
    import concourse.bass as bass
    from concourse import mybir
    from concourse.bass_types import AP
    from contextlib import ExitStack

    f32 = mybir.dt.float32
    bf16 = mybir.dt.bfloat16
    i8 = mybir.dt.int8

    n_cores = len(offs_cores)
    n_mat = l * b_loc
    n_pair = n_mat // 2
    assert l % tsz == 0 and tsz % 32 == 0
    assert g_slots % 4 == 0
    n_band = l // tsz
    n_grp = n_band * b_loc          # output DMA groups (band-major)
    n_psum = 4

    nc = bass.Bass("TRN2")
    fsa8 = nc.dram_tensor("fsa8", [v * S, ROW_BYTES], i8, kind="ExternalInput")
    out = nc.dram_tensor("out", [b_loc, l, S], bf16, kind="ExternalOutput")
    if instrument:
        mark_d = nc.dram_tensor("marker_out", [1, maxtick], f32, kind="ExternalOutput")

    tab_dt = bf16 if TBYTES == 2 else mybir.dt.float8e4

    with ExitStack() as stack:
        gbuf = stack.enter_context(nc.sbuf_tensor("gbuf", [128, g_slots, S], tab_dt))
        gbuf8 = gbuf.bitcast(i8)    # [128, g_slots, ROW_BYTES]
        h_hist = stack.enter_context(nc.sbuf_tensor("h_hist", [128, l, b_loc], bf16))
        h0 = stack.enter_context(nc.sbuf_tensor("h0", [128, 1], bf16))
        # one stbuf slot per output group: DVE never waits on output DMAs
        stbuf = stack.enter_context(nc.sbuf_tensor("stbuf", [tsz, n_grp, S], bf16))
        ph = stack.enter_context(nc.psum_tensor("ph", [128, n_psum, 512], f32))
        # 8 rotating completion sems per gather engine: a wait on class k%8
        # at count k//8+1 plus per-SDMA-engine ring-FIFO order proves every
        # op <= k fully landed (a plain summed sem can transiently reach
        # 16*K with op K still in flight when engines skew by one op)
        NGS = 8
        sp_gsems = [
            stack.enter_context(nc.semaphore(f"sp_gsem{r}")) for r in range(NGS)
        ]
        act_gsems = [
            stack.enter_context(nc.semaphore(f"act_gsem{r}")) for r in range(NGS)
        ]
        pe_h_sem = stack.enter_context(nc.semaphore("pe_h_sem"))
        dve_sem = stack.enter_context(nc.semaphore("dve_sem"))
        tr_sem = stack.enter_context(nc.semaphore("tr_sem"))
        so_sem = stack.enter_context(nc.semaphore("so_sem"))
        if instrument:
            marker = stack.enter_context(nc.sbuf_tensor("marker", [1, maxtick], f32))
            ms_sem = stack.enter_context(nc.semaphore("ms_sem"))
        block = stack.enter_context(nc.Block())

        def pair_in_ap(o1, o2):
            """3-dim AP covering matrices at byte offsets o1, o2 (in stream
            order); returns (ap, reversed)."""
            lo, hi, rev = (o1, o2, False) if o2 >= o1 else (o2, o1, True)
            return AP(
                tensor=fsa8,
                offset=int(lo),
                ap=[[ROW_BYTES, S], [int(hi - lo), 2], [1, ROW_BYTES]],
                dep_tracking_offset=0,
            ), rev

        def pair_out_ap(s, rev):
            if not rev:
                return gbuf8[:, s : s + 2, :]
            return AP(
                tensor=gbuf8,
                offset=(s + 1) * ROW_BYTES,
                ap=[[g_slots * ROW_BYTES, 128], [-ROW_BYTES, 2], [1, ROW_BYTES]],
                dep_tracking_offset=0,
            )

        def gather_stream(eng, sems, parity):
            """Pair ops j = parity, parity+2, ... on engine eng."""
            pid = eng.partition_id()
            for c in range(n_cores):
                offs = offs_cores[c]
                with eng.If(pid == c):
                    for k in range(n_pair // 2):
                        j = 2 * k + parity
                        p0 = 2 * j          # first stream position
                        if p0 + 1 >= g_slots:
                            eng.wait_ge(pe_h_sem, (p0 + 1 - g_slots) // 8 + 1)
                        sap, rev = pair_in_ap(offs[p0], offs[p0 + 1])
                        eng.dma_start(
                            out=pair_out_ap(p0 % g_slots, rev), in_=sap
                        ).then_inc(sems[k % NGS], 16)

        @block.sync
        def _(sync):
            gather_stream(sync, sp_gsems, 0)
            for g in range(n_grp):
                b = g % b_loc
                tb = g // b_loc
                sync.wait_ge(tr_sem, g + 1)
                sync.dma_start(
                    out=out[b, tb * tsz : (tb + 1) * tsz, :],
                    in_=stbuf[:, g, :],
                ).then_inc(so_sem, 16)
            if instrument:
                sync.wait_ge(so_sem, 16 * n_grp)
                sync.dma_start(out=mark_d[:, :], in_=marker[:, :]).then_inc(
                    ms_sem, 16
                )

        @block.scalar
        def _(scalar):
            gather_stream(scalar, act_gsems, 1)

        @block.tensor
        def _(tensor):
            for t in range(l):
                # last gather op needed for step t is k = 2t+1 on each engine
                k = 2 * t + 1
                tensor.wait_ge(sp_gsems[k % NGS], 16 * (k // NGS + 1))
                tensor.wait_ge(act_gsems[k % NGS], 16 * (k // NGS + 1))
                tensor.wait_ge(dve_sem, t + 1)
                mm = None
                for b in range(b_loc):
                    n = t * b_loc + b
                    rhs = h0[:, 0:1] if t == 0 else h_hist[:, t - 1, b : b + 1]
                    mm = tensor.matmul(
                        out=ph[:, t % n_psum, b : b + 1],
                        lhsT=gbuf[:, n % g_slots, :],
                        rhs=rhs,
                        start=True,
                        stop=True,
                    )
                mm.then_inc(pe_h_sem, 1)

        @block.vector
        def _(vector):
            vector.memset(h0[:, :], 0.0)
            vector.memset(h0[:1, :], 1.0).then_inc(dve_sem, 1)

            n_jb = tsz // 32        # 32-blocks along t within a band
            n_ib = S // 32          # 32-blocks along state dim
            per_band = b_loc * n_jb * n_ib  # == tsz steps per band: 1/step

            def do_transpose(tb, k):
                b = k // (n_jb * n_ib)
                jb = (k % (n_jb * n_ib)) // n_ib
                ib = k % n_ib
                g = tb * b_loc + b
                tr = vector.transpose(
                    out=stbuf[32 * jb : 32 * (jb + 1), g, 32 * ib : 32 * (ib + 1)],
                    in_=h_hist[
                        32 * ib : 32 * (ib + 1),
                        tb * tsz + 32 * jb : tb * tsz + 32 * (jb + 1),
                        b,
                    ],
                )
                if k % (n_jb * n_ib) == n_jb * n_ib - 1:
                    tr.then_inc(tr_sem, 1)

            for t in range(l):
                vector.wait_ge(pe_h_sem, t + 1)
                vector.tensor_scalar(
                    h_hist[:, t, :],
                    ph[:, t % n_psum, 0:b_loc],
                    -10.0,
                    10.0,
                    mybir.AluOpType.max,
                    mybir.AluOpType.min,
                ).then_inc(dve_sem, 1)
                tb = t // tsz - 1
                if tb >= 0:
                    do_transpose(tb, t % tsz)
            for k in range(per_band):
                do_transpose(n_band - 1, k)

        if instrument:

            @block.gpsimd
            def _(gpsimd):
                gpsimd.memset(marker[:, :], 0.0)
                for i in range(maxtick):
                    gpsimd.nop(cycle_cnt=tick_cyc, nofuse=True)
                    gpsimd.memset(marker[:1, i : i + 1], 1.0)

    return nc


def make_offs(tok_c):
    """tok_c: [b_loc, l] ints -> flat [l*b_loc] int64 byte offsets, stream
    order n = t*b_loc + b."""
    return (tok_c.T.astype(np.int64) * MAT_BYTES).reshape(-1)


def _prep_fsa(fsa_tensor):
    import ml_dtypes

    np_dt = ml_dtypes.bfloat16 if TBYTES == 2 else ml_dtypes.float8_e4m3
    fsa_t = np.asarray(fsa_tensor, dtype=np.float32).astype(np_dt)
    return np.ascontiguousarray(fsa_t).view(np.int8).reshape(V * S, ROW_BYTES)


def run(input, lengths, fsa_tensor):
    from concourse.bass_utils import run_bass_kernel_spmd

    tok = np.asarray(input)
    fsa8 = _prep_fsa(fsa_tensor)
    offs_cores = [
        make_offs(tok[c * B_LOC : (c + 1) * B_LOC]) for c in range(N_CORES)
    ]
    nc = build_kernel(offs_cores)
    in_maps = [{"fsa8": fsa8} for _ in range(N_CORES)]
    res = run_bass_kernel_spmd(nc, in_maps, core_ids=list(range(N_CORES)))
    out = np.concatenate(
        [r["out"].astype(np.float32) for r in res.results], axis=0
    )
    return out, res


def kernel(input, lengths, fsa_tensor):
    out, _ = run(input, lengths, fsa_tensor)
    return out
